# revision 56
# baseline (speedup 1.0000x reference)
#!/usr/bin/env python
"""Tensor-parallel fused attention kernel for Trainium2 (8 NeuronCores).

Sharding: one KV head (+ its 4 grouped Q heads) per core.
 - w_qkv column-parallel (each core computes its 768 qkv rows)
 - attention fully local per core (its heads)
 - RMSNorm/quant per-token stats via a tiny per-chunk AllGather whose
   consumer chain is deferred one chunk (engine queues are in-order, so
   this keeps the collective latency off the critical path)
 - w_o row-parallel; partial [DIM, T] outputs (f16) summed on host.

V2 layout rules (vs the V1 baseline):
 - NO DRAM round-trips for transposes/broadcasts: partition reductions go
   through PE transposes, row->all-partition broadcasts through K=1
   matmuls with a ones [1,128] stationary.
 - elementwise work split across DVE / Activation / Pool engines.
 - w_o is streamed once at the end; qz for all T resident as bf16
   integers (exact: |qz_int| <= 127), the per-token dequant scale c2 is
   factored out of the matmul and applied to the [DIM,T] output tiles.
 - y partials written as f16 (halves the output DMA).

Per-core layouts (everything transposed, d-major):
 - xT [DIM, T]; per-sub xq [128, 4, CT] f32r (transient)
 - q/k head-dim rows permuted (evens then odds) so RoPE is a half-swap
   done with partition-offset DVE ops (no DMA); v unpermuted.
 - K_sb [128, T] f32r, V_sb token-major [128, T/128, 128] f32r
 - z (attn out * rms_w) [128, 4, CT] f32 double-buffered
 - wo DRAM [128, DIM, 4] bf16 so each [128,128] stationary slice is
   read with 1KB runs.
"""
import sys
sys.path.insert(0, '/opt/trn_rl_repo')

import numpy as np
from contextlib import ExitStack

import concourse.bass as bass
import concourse.bacc as bacc_mod
import concourse.tile as tile
import concourse.mybir as mybir

F32 = mybir.dt.float32
F32R = mybir.dt.float32r
F16 = mybir.dt.float16
BF16 = mybir.dt.bfloat16
AF = mybir.ActivationFunctionType
OP = mybir.AluOpType
AX = mybir.AxisListType

DIM = 4096
NH = 32
NKV = 8
HPG = 4          # q heads per kv head (per core)
HD = 128
NCORES = 8
JQ = HPG * HD    # 512 local q rows
JL = JQ + 2 * HD # 768 local qkv rows
CT = 256         # tokens per chunk
KB = CT // 128   # key blocks (128 tokens) per chunk
NSUB = 8         # x sub-loads per chunk (4 d-chunks each)
DSUB = 4         # d-chunks per sub-load
THETA = 500000.0
EPS = 1e-5
SCALE = float(HD) ** -0.5
MAGIC = float(3 << 22)  # 12582912.0 = 1.5*2^23, ulp 1.0 range
DCH = DIM // 128  # 32 d-chunks


def build_kernel(T=2048, use_cc=True, debug=False):
    NCH = T // CT
    nc = bacc_mod.Bacc("TRN2", num_devices=NCORES)
    if debug:
        dbg_qk = nc.dram_tensor("dbg_qk", [JL, T], F32, kind="ExternalOutput")
        dbg_z = nc.dram_tensor("dbg_z", [JQ, T], F32, kind="ExternalOutput")

    # ---- I/O -------------------------------------------------------------
    xt_d = nc.dram_tensor("xt", [DIM, T], F32, kind="ExternalInput")
    xpart_d = nc.dram_tensor("xpart", [JQ, T], F32, kind="ExternalInput")
    wq_d = nc.dram_tensor("wq", [DIM, JL], F32R, kind="ExternalInput")
    wo_d = nc.dram_tensor("wo", [128, DIM, HPG], BF16, kind="ExternalInput")
    rms_d = nc.dram_tensor("rms", [JQ], F32, kind="ExternalInput")
    cosf_d = nc.dram_tensor("cosf", [128, T], F32, kind="ExternalInput")
    sinf_d = nc.dram_tensor("sinf", [128, T], F32, kind="ExternalInput")
    mask_d = nc.dram_tensor("maskt", [128, KB, CT], BF16, kind="ExternalInput")
    id_d = nc.dram_tensor("ident", [128, 128], F32R, kind="ExternalInput")
    onesc_d = nc.dram_tensor("onesc", [128, 1], F32R, kind="ExternalInput")
    onesr_d = nc.dram_tensor("onesr", [1, 128], F32R, kind="ExternalInput")
    yt_d = nc.dram_tensor("yt", [DIM, T], F16, kind="ExternalOutput")
    c2o_d = nc.dram_tensor("c2o", [T // CT, 128, KB], F32, kind="ExternalOutput")

    with ExitStack() as ctx:
        tc = ctx.enter_context(tile.TileContext(nc))
        persist = ctx.enter_context(tc.tile_pool(name="persist", bufs=1))
        work = ctx.enter_context(tc.tile_pool(name="work", bufs=2))
        dram = ctx.enter_context(tc.tile_pool(name="dram", bufs=1, space="DRAM"))
        # PSUM pools: 8 banks x 2KB/partition; slots are bank-granular, and a
        # bank supports only ONE OPEN accumulation group at a time (verified
        # on HW: interleaving two open groups corrupts the first), though
        # sequential groups + sub-view hazards are fine.
        # pq0-5: 6 qkv accumulators, one bank each. The same six families
        # are reused (by tag) in the attention phase -- sp rotates pq0/pq1
        # (kb parity), pv rotates pq2/pq3 (head parity), lp uses pq4,
        # ivl broadcasts pq5 -- and yp reuses them in the tail.
        # misc (transposes, small broadcasts), bufs=2             = 2 banks
        mm_ps = ctx.enter_context(tc.tile_pool(name="mm_ps", bufs=1, space="PSUM"))
        misc_ps = ctx.enter_context(tc.tile_pool(name="misc_ps", bufs=2, space="PSUM"))

        # ---- persistent tiles -------------------------------------------
        wq_sb = persist.tile([128, DCH, JL], F32R)
        for pc in range(NSUB):
            nc.sync.dma_start(
                wq_sb[:, pc * DSUB:(pc + 1) * DSUB, :],
                wq_d.ap()[pc * DSUB * 128:(pc + 1) * DSUB * 128, :]
                .rearrange("(dc p) j -> p dc j", p=128))
        K_sb = persist.tile([128, T], F32R)
        V_sb = persist.tile([128, T // 128, HD], F32R)
        qz_all = persist.tile([128, HPG, T], BF16)   # round(z*qf): ints <=127
        rms_sb = persist.tile([128, HPG], F32)
        nc.sync.dma_start(rms_sb[:], rms_d.ap().rearrange("(h p) -> p h", p=128))
        id_sb = persist.tile([128, 128], F32R)
        nc.sync.dma_start(id_sb[:], id_d.ap())
        onesc_sb = persist.tile([128, 1], F32R)
        nc.sync.dma_start(onesc_sb[:], onesc_d.ap())
        onesr_sb = persist.tile([1, 128], F32R)
        nc.sync.dma_start(onesr_sb[:], onesr_d.ap())
        mask_sb = persist.tile([128, KB, CT], BF16)
        nc.sync.dma_start(mask_sb[:], mask_d.ap())
        eps_sb = persist.tile([128, 1], F32)
        nc.vector.memset(eps_sb[:], EPS)

        def bcast_row(row_ap, name):
            """[1, 128] f32 row -> [128, 128] PSUM tile via K=1 matmul."""
            ps = misc_ps.tile([128, 128], F32, name=name, tag="misc")
            nc.tensor.matmul(ps[:], onesr_sb[:].bitcast(F32), row_ap,
                             start=True, stop=True)
            return ps

        def post_chain(c, z_c, statg):
            """Deferred consumer of chunk c's stats AllGather: global stats,
            quant coefficients, and z -> qz_all/c2_all. Emitted one chunk
            later so the in-order engine queues never stall on the CC."""
            t0 = c * CT
            tsl = slice(t0, t0 + CT)
            gst = work.tile([128, NCORES, KB, 2], F32, name="gst", bufs=2)
            nc.sync.dma_start(gst[:],
                              statg[:].rearrange("c p b s -> p c b s"))
            sst = work.tile([128, KB], F32, name="sst", bufs=2)
            nc.vector.tensor_reduce(
                sst[:], gst[:, :, :, 0].rearrange("p c b -> p b c"),
                axis=AX.X, op=OP.add)
            mxt = work.tile([128, KB], F32, name="mxt", bufs=2)
            nc.vector.tensor_reduce(
                mxt[:], gst[:, :, :, 1].rearrange("p c b -> p b c"),
                axis=AX.X, op=OP.max)
            # r = 1/sqrt(ssq/DIM + eps); rg = clip(r*gmax); qf = 127*r/rg
            rc = work.tile([128, KB], F32, name="rc", bufs=2)
            nc.scalar.activation(rc[:], sst[:], AF.Sqrt, scale=1.0 / DIM,
                                 bias=eps_sb[:])
            nc.vector.reciprocal(rc[:], rc[:])
            rg = work.tile([128, KB], F32, name="rg", bufs=2)
            nc.vector.tensor_tensor(rg[:], rc[:], mxt[:], OP.mult)
            nc.vector.tensor_scalar_max(rg[:], rg[:], 1e-5)
            qc4 = work.tile([128, 2 * KB], F32, name="qc4", bufs=2)
            rr = work.tile([128, KB], F32, name="rr", bufs=2)
            nc.vector.reciprocal(rr[:], rg[:])
            nc.vector.scalar_tensor_tensor(qc4[:, 0:KB], rr[:], 127.0,
                                           rc[:], OP.mult, OP.mult)
            nc.vector.tensor_scalar_mul(qc4[:, KB:], rg[:], 1.0 / 127.0)
            # c2 goes to the host (applied to the summed partials there)
            nc.sync.dma_start(c2o_d.ap()[c], qc4[:, KB:])
            qt_ps = misc_ps.tile([1, KB * 128], F32, name="qt_ps", tag="misc")
            for j in range(KB):
                nc.tensor.transpose(qt_ps[0:1, j * 128:(j + 1) * 128],
                                    qc4[:, j:j + 1], id_sb[:].bitcast(F32))
            qrow = work.tile([1, KB * 128], F32, name="qrow", bufs=1)
            nc.vector.tensor_copy(qrow[:], qt_ps[:])
            qf_bc = work.tile([128, HPG, CT], F32, name="qf_bc", bufs=1)
            for tb in range(KB):
                ps = bcast_row(qrow[0:1, tb * 128:(tb + 1) * 128], "qfb_ps")
                for hh in range(HPG):
                    nc.scalar.activation(qf_bc[:, hh, tb * 128:(tb + 1) * 128],
                                         ps[:], AF.Copy)
            # quantize: qz_int = round(z*qf) (exact integers in bf16);
            # z*qf written in place (last use of z_c)
            qzv = qz_all[:, :, tsl]
            nc.vector.tensor_tensor(z_c[:], z_c[:], qf_bc[:], OP.mult)
            nc.scalar.activation(z_c[:], z_c[:], AF.Copy, bias=MAGIC)
            nc.scalar.activation(qzv, z_c[:], AF.Copy, bias=-MAGIC)

        def pass1_pre(c):
            """Local abs-max partials for chunk c + AllGather issue.
            Emitted TWO chunks ahead of the heavy block, so the collective
            completes during the previous block. The first two chunks
            compute the full max locally instead (no collective rendezvous
            on the cold-start critical path)."""
            t0 = c * CT
            tsl = slice(t0, t0 + CT)
            if c < 2:
                mloc = work.tile([128, CT], F32, name="mloc", bufs=1)
                for sub in range(NSUB):
                    xsl = work.tile([128, DSUB, CT], F32, name="xs", bufs=2)
                    nc.sync.dma_start(
                        xsl[:],
                        xt_d.ap()[sub * DSUB * 128:(sub + 1) * DSUB * 128, tsl]
                        .rearrange("(d p) t -> p d t", p=128))
                    msl = work.tile([128, CT], F32, name="msl", bufs=1)
                    nc.vector.tensor_reduce(
                        msl[:], xsl[:].rearrange("p d t -> p t d"), axis=AX.X,
                        op=OP.max, apply_absolute_value=True)
                    if sub == 0:
                        nc.gpsimd.tensor_copy(mloc[:], msl[:])
                    else:
                        nc.vector.tensor_tensor(mloc[:], mloc[:], msl[:], OP.max)
                mcolL = work.tile([128, KB], F32, name="mcolL", bufs=1)
                for tb in range(KB):
                    mt = misc_ps.tile([128, 128], F32, name="mt", tag="misc")
                    nc.tensor.transpose(
                        mt[:], mloc[:, tb * 128:(tb + 1) * 128],
                        id_sb[:].bitcast(F32))
                    nc.vector.tensor_reduce(mcolL[:, tb:tb + 1], mt[:],
                                            axis=AX.X, op=OP.max)
                return ("local", mcolL)
            # Each core reduces only ITS 512 dims of x (the same rows its
            # attention heads own); partial maxima are exchanged via a tiny
            # AllGather that hides under the previous chunk's attention.
            xs = work.tile([128, DSUB, CT], F32, name="xs", bufs=2)
            nc.sync.dma_start(
                xs[:], xpart_d.ap()[:, tsl].rearrange("(d p) t -> p d t", p=128))
            mpart = work.tile([128, CT], F32, name="mpart", bufs=1)
            nc.vector.tensor_reduce(
                mpart[:], xs[:].rearrange("p d t -> p t d"), axis=AX.X,
                op=OP.max, apply_absolute_value=True)
            mcolp = work.tile([128, KB], F32, name="mcolp", bufs=2)
            for tb in range(KB):
                mt = misc_ps.tile([128, 128], F32, name="mt", tag="misc")
                nc.tensor.transpose(
                    mt[:], mpart[:, tb * 128:(tb + 1) * 128],
                    id_sb[:].bitcast(F32))
                nc.vector.tensor_reduce(mcolp[:, tb:tb + 1], mt[:], axis=AX.X,
                                        op=OP.max)
            mpd = dram.tile([128, KB], F32, name=f"mpd{c}")
            nc.sync.dma_start(mpd[:], mcolp[:])
            mpg = dram.tile([NCORES, 128, KB], F32, name=f"mpg{c}")
            if use_cc:
                nc.gpsimd.collective_compute(
                    "AllGather", OP.bypass, replica_groups=[list(range(NCORES))],
                    ins=[mpd[:].opt()], outs=[mpg[:].opt()])
            else:
                for cc in range(NCORES):
                    nc.sync.dma_start(mpg[cc], mpd[:])
            return ("cc", mpg)

        def pass1_post(c, token):
            """Consume chunk c's abs-max (AllGather or local) -> s/sinv
            broadcasts. Emitted one chunk ahead of the heavy block."""
            kind, val = token
            if kind == "cc":
                mg = work.tile([128, NCORES, KB], F32, name="mg", bufs=2)
                nc.sync.dma_start(mg[:], val[:].rearrange("c p b -> p c b"))
                mcol = work.tile([128, KB], F32, name="mcol", bufs=2)
                nc.vector.tensor_reduce(
                    mcol[:], mg[:].rearrange("p c b -> p b c"), axis=AX.X,
                    op=OP.max)
            else:
                mcol = val
            nc.vector.tensor_scalar_max(mcol[:], mcol[:], 1e-5)
            scoef = work.tile([128, 2 * KB], F32, name="scoef", bufs=2)
            rec = work.tile([128, KB], F32, name="rec", bufs=1)
            nc.vector.reciprocal(rec[:], mcol[:])
            nc.vector.tensor_scalar_mul(scoef[:, 0:KB], rec[:], 127.0)
            nc.vector.tensor_scalar_mul(scoef[:, KB:], mcol[:], 1.0 / 127.0)
            st_ps = misc_ps.tile([1, 2 * KB * 128], F32, name="st_ps", tag="misc")
            for j in range(2 * KB):
                nc.tensor.transpose(st_ps[0:1, j * 128:(j + 1) * 128],
                                    scoef[:, j:j + 1], id_sb[:].bitcast(F32))
            srow = work.tile([1, 2 * KB * 128], F32, name="srow", bufs=2)
            nc.vector.tensor_copy(srow[:], st_ps[:])
            # broadcast rows across partitions (PSUM), then copy to SBUF
            s_bc = work.tile([128, DSUB, CT], F32, name="s_bc", bufs=2)
            sinv_bc = work.tile([128, CT], F32, name="sinv_bc", bufs=2)
            for tb in range(KB):
                ps = bcast_row(srow[0:1, tb * 128:(tb + 1) * 128], "sbc_ps")
                for d in range(DSUB):
                    nc.scalar.activation(s_bc[:, d, tb * 128:(tb + 1) * 128],
                                         ps[:], AF.Copy)
                ps2 = bcast_row(srow[0:1, (KB + tb) * 128:(KB + tb + 1) * 128],
                                "svbc_ps")
                nc.scalar.activation(sinv_bc[:, tb * 128:(tb + 1) * 128], ps2[:],
                                     AF.Copy)
            return s_bc, sinv_bc

        def qkv_part(c, s_bc, sinv_bc):
            t0 = c * CT
            tsl = slice(t0, t0 + CT)
            cos_ch = work.tile([128, CT], F32, name="cos_ch", bufs=2)
            nc.sync.dma_start(cos_ch[:], cosf_d.ap()[:, tsl])
            sin_ch = work.tile([128, CT], F32, name="sin_ch", bufs=2)
            nc.sync.dma_start(sin_ch[:], sinf_d.ap()[:, tsl])

            # ---- pass 2 over x: quantize + QKV projection ---------------
            pq = [mm_ps.tile([128, CT], F32, name=f"pq{jc}", tag=f"pq{jc}")
                  for jc in range(6)]
            for sub in range(NSUB):
                xs2 = work.tile([128, DSUB, CT], F32, name="xs", bufs=2)
                nc.sync.dma_start(
                    xs2[:], xt_d.ap()[sub * DSUB * 128:(sub + 1) * DSUB * 128, tsl]
                    .rearrange("(d p) t -> p d t", p=128))
                xq = work.tile([128, DSUB, CT], F32R, name="xq", bufs=2)
                nc.vector.tensor_tensor(xs2[:], xs2[:], s_bc[:], OP.mult)
                nc.scalar.activation(xs2[:], xs2[:], AF.Copy, bias=MAGIC)
                nc.scalar.activation(xq[:], xs2[:], AF.Copy, bias=-MAGIC)
                for jc in range(6):
                    for di in range(DSUB):
                        dc = sub * DSUB + di
                        nc.tensor.matmul(
                            pq[jc][:], wq_sb[:, dc, jc * 128:(jc + 1) * 128],
                            xq[:, di, :], start=(dc == 0), stop=(dc == DCH - 1))

            # ---- sinv scale + split into q / k / v ----------------------
            q4 = work.tile([128, HPG, CT], F32R, name="q4", bufs=1)
            k_tmp = work.tile([128, CT], F32, name="k_tmp", bufs=1)
            v_tmp = work.tile([128, CT], F32R, name="v_tmp", bufs=2)
            for jc in range(6):
                dst = (q4[:, jc, :] if jc < HPG
                       else (k_tmp[:] if jc == HPG else v_tmp[:]))
                nc.vector.tensor_tensor(dst, pq[jc][:], sinv_bc[:], OP.mult)

            # ---- RoPE (half-swap via small SBUF-SBUF DMA) ---------------
            def rope(dst, src):
                xsw = work.tile([128, CT], F32, name="xsw", bufs=2)
                nc.sync.dma_start(xsw[0:64, :], src[64:128, :])
                nc.sync.dma_start(xsw[64:128, :], src[0:64, :])
                nc.vector.tensor_tensor(xsw[:], xsw[:], sin_ch[:], OP.mult)
                a = work.tile([128, CT], F32, name="rcos", bufs=1)
                nc.vector.tensor_tensor(a[:], src, cos_ch[:], OP.mult)
                nc.vector.tensor_tensor(dst, a[:], xsw[:], OP.add)

            for h in range(HPG):
                rope(q4[:, h, :], q4[:, h, :].bitcast(F32))
            rope(K_sb[:, tsl], k_tmp[:])
            if debug:
                for h in range(HPG):
                    nc.sync.dma_start(dbg_qk.ap()[h * 128:(h + 1) * 128, tsl],
                                      q4[:, h, :].bitcast(F32))
                nc.sync.dma_start(dbg_qk.ap()[JQ:JQ + HD, tsl],
                                  K_sb[:, tsl].bitcast(F32))
                nc.sync.dma_start(dbg_qk.ap()[JQ + HD:, tsl],
                                  v_tmp[:].bitcast(F32))

            # ---- V transpose to token-major -----------------------------
            for tb in range(KB):
                vt = misc_ps.tile([128, 128], F32R, name="vt", tag="misc")
                nc.tensor.transpose(vt[:], v_tmp[:, tb * 128:(tb + 1) * 128],
                                    id_sb[:])
                nc.vector.tensor_copy(V_sb[:, c * KB + tb, :], vt[:])

            return q4

        def attn_part(c, q4):
            t0 = c * CT
            tsl = slice(t0, t0 + CT)
            # ---- attention (per head; normalization deferred one head) --
            nkb = KB * (c + 1)
            z_c = work.tile([128, HPG, CT], F32, name="z_c", bufs=2)
            ssqmx = work.tile([128, KB, 2], F32, name="ssqmx", bufs=2)

            def finish_head(h, pv, lrow):
                # PE-broadcast of 1/l, then z = pv/l, ssq stats, rms scale
                ivl_ps = mm_ps.tile([128, CT], F32, name="ivl_ps", tag="pq5")
                nc.tensor.matmul(ivl_ps[:], onesr_sb[:].bitcast(F32), lrow[:],
                                 start=True, stop=True)
                ivl_sb = work.tile([128, CT], F32, name="ivl_sb", bufs=1)
                nc.scalar.activation(ivl_sb[:], ivl_ps[:], AF.Copy)
                nc.vector.tensor_tensor(z_c[:, h, :], pv[:], ivl_sb[:], OP.mult)
                sq = work.tile([128, CT], F32R, name="sq", bufs=1)
                nc.scalar.activation(sq[:], z_c[:, h, :], AF.Square)
                for tb in range(KB):
                    sqt = misc_ps.tile([128, 128], F32R, name="sqt", tag="misc")
                    nc.tensor.transpose(sqt[:], sq[:, tb * 128:(tb + 1) * 128],
                                        id_sb[:])
                    if h == 0:
                        nc.vector.tensor_reduce(ssqmx[:, tb, 0:1], sqt[:],
                                                axis=AX.X, op=OP.add)
                    else:
                        hcol = work.tile([128, 1], F32, name="hcol", bufs=2)
                        nc.vector.tensor_reduce(hcol[:], sqt[:], axis=AX.X,
                                                op=OP.add)
                        nc.vector.tensor_tensor(ssqmx[:, tb, 0:1],
                                                ssqmx[:, tb, 0:1], hcol[:],
                                                OP.add)
                # z = out * rms_w (per-partition scalar); in-place after Square
                nc.vector.tensor_scalar(z_c[:, h, :], z_c[:, h, :],
                                        rms_sb[:, h:h + 1], None, OP.mult)
                if debug:
                    nc.sync.dma_start(dbg_z.ap()[h * 128:(h + 1) * 128, tsl],
                                      z_c[:, h, :])

            prev_head = None
            for h in range(HPG):
                pv = mm_ps.tile([128, CT], F32, name="pv", tag=f"pq{2 + h % 2}")
                lp = mm_ps.tile([1, CT], F32, name="lp", tag="pq4")
                for kp in range(nkb // 2):
                    # two key blocks share one score bank and a single exp
                    sp = mm_ps.tile([128, 2, CT], F32, name="sp",
                                    tag=f"pq{kp % 2}")
                    for i in range(2):
                        kb = 2 * kp + i
                        nc.tensor.matmul(sp[:, i, :],
                                         K_sb[:, kb * 128:(kb + 1) * 128],
                                         q4[:, h, :], start=True, stop=True,
                                         skip_group_check=True)
                    P = work.tile([128, 2, CT], F32R, name="P", bufs=2)
                    nc.scalar.activation(P[:], sp[:], AF.Exp, scale=SCALE)
                    if kp == nkb // 2 - 1:
                        nc.gpsimd.tensor_tensor(P[:], P[:].bitcast(F32),
                                                mask_sb[:], OP.mult)
                    for i in range(2):
                        kb = 2 * kp + i
                        nc.tensor.matmul(pv[:], V_sb[:, kb, :], P[:, i, :],
                                         start=(kb == 0), stop=(kb == nkb - 1))
                        nc.tensor.matmul(lp[:], onesc_sb[:], P[:, i, :],
                                         start=(kb == 0), stop=(kb == nkb - 1))
                lrow = work.tile([1, CT], F32, name="lrow", bufs=2)
                nc.vector.reciprocal(lrow[:], lp[:])
                if prev_head is not None:
                    finish_head(*prev_head)
                prev_head = (h, pv, lrow)
            finish_head(*prev_head)

            # ---- per-token max|z| over local dims -----------------------
            mz = work.tile([128, CT], F32, name="mz", bufs=2)
            nc.vector.tensor_reduce(
                mz[:], z_c[:].rearrange("p h t -> p t h"),
                axis=AX.X, op=OP.max, apply_absolute_value=True)
            for tb in range(KB):
                mzt = misc_ps.tile([128, 128], F32, name="mzt", tag="misc")
                nc.tensor.transpose(mzt[:], mz[:, tb * 128:(tb + 1) * 128],
                                    id_sb[:].bitcast(F32))
                nc.vector.tensor_reduce(ssqmx[:, tb, 1:2], mzt[:], axis=AX.X,
                                        op=OP.max)

            # ---- stats collective (consumed one chunk later) ------------
            statd = dram.tile([128, KB, 2], F32, name=f"statd{c}")
            nc.sync.dma_start(statd[:], ssqmx[:])
            statg = dram.tile([NCORES, 128, KB, 2], F32, name=f"statg{c}")
            if use_cc:
                nc.gpsimd.collective_compute(
                    "AllGather", OP.bypass, replica_groups=[list(range(NCORES))],
                    ins=[statd[:].opt()], outs=[statg[:].opt()])
            else:
                for cc in range(NCORES):
                    nc.sync.dma_start(statg[cc], statd[:])
            return z_c, statg

        # ---- software-pipelined main loop ------------------------------
        # iteration i emits, in order: QKV(i-2) (so xq production leads the
        # PE), max-collective issue(i), collective consume(i-1) -> s/sinv,
        # attention(i-2), stats chain(i-3).
        pending = None
        mpg_carry = None
        sc_carry = None
        for i in range(NCH + 2):
            mpg_nxt = pass1_pre(i) if i < NCH else None
            sc_nxt = pass1_post(i - 1, mpg_carry) if 1 <= i <= NCH else None
            if i >= 2:
                q4h = qkv_part(i - 2, *sc_carry)
                z_c, statg = attn_part(i - 2, q4h)
                if pending is not None:
                    post_chain(*pending)
                pending = (i - 2, z_c, statg)
            mpg_carry = mpg_nxt
            sc_carry = sc_nxt

        post_chain(*pending)

        # ---- tail: single streamed w_o pass ------------------------------
        NTS = T // CT
        for ic in range(DCH):
            wo_t = work.tile([128, 128, HPG], BF16, name="wo_t", bufs=2)
            nc.sync.dma_start(wo_t[:], wo_d.ap()[:, ic * 128:(ic + 1) * 128, :])
            for half in range(4):
                yrow = work.tile([128, T // 4], F16, name="yrow", bufs=2)
                nts_h = NTS // 4
                yps = [mm_ps.tile([128, CT], F32, name=f"yp{tsi}",
                                  tag=f"pq{2 * (half % 2) + tsi}")
                       for tsi in range(nts_h)]
                for jc in range(HPG):
                    for tsi in range(nts_h):
                        ts = half * nts_h + tsi
                        nc.tensor.matmul(
                            yps[tsi][:], wo_t[:, :, jc],
                            qz_all[:, jc, ts * CT:(ts + 1) * CT],
                            start=(jc == 0), stop=(jc == HPG - 1))
                # y = wo @ qz_int (the c2 dequant is applied on the host)
                for tsi in range(nts_h):
                    dst = yrow[:, tsi * CT:(tsi + 1) * CT]
                    if tsi % 2 == 0:
                        nc.vector.tensor_copy(dst, yps[tsi][:])
                    else:
                        nc.scalar.activation(dst, yps[tsi][:], AF.Copy)
                nc.sync.dma_start(
                    yt_d.ap()[ic * 128:(ic + 1) * 128,
                              half * (T // 4):(half + 1) * (T // 4)],
                    yrow[:])
    nc.compile()
    return nc


# ======================= host-side preparation ==========================

def _rope_tables(T):
    import jax
    import jax.numpy as jnp
    cpu = jax.devices("cpu")[0]
    with jax.default_device(cpu):
        inv = THETA ** (-jnp.arange(0, HD, 2, dtype=jnp.float32) / HD)
        pos = jnp.arange(T, dtype=jnp.float32)
        ang = pos[None, :] * inv[:, None]          # [64, T]
        cos = np.asarray(jnp.cos(ang), dtype=np.float32)
        sin = np.asarray(jnp.sin(ang), dtype=np.float32)
    cosf = np.concatenate([cos, cos], axis=0)       # [128, T]
    sinf = np.concatenate([-sin, sin], axis=0)
    return np.ascontiguousarray(cosf), np.ascontiguousarray(sinf)


def _perm_rope():
    """head-dim permutation: evens then odds."""
    return np.concatenate([np.arange(0, HD, 2), np.arange(1, HD, 2)])


def make_inputs(x, w_qkv, w_o, rms_w, T=2048):
    """Build the 8 per-core input dicts from full inputs."""
    import ml_dtypes
    perm = _perm_rope()
    cosf, sinf = _rope_tables(T)
    mask = np.zeros((128, KB, CT), dtype=np.float32)
    kt = np.arange(128)[:, None]
    qt = np.arange(CT)[None, :]
    for d in range(KB):
        mask[:, d, :] = (kt + 128 * d <= qt)
    maskb = mask.astype(ml_dtypes.bfloat16)
    ident = np.eye(128, dtype=np.float32)
    onesc = np.ones((128, 1), dtype=np.float32)
    onesr = np.ones((1, 128), dtype=np.float32)

    wq_full = w_qkv[:NH * HD].reshape(NKV, HPG, HD, DIM)
    wk_full = w_qkv[NH * HD:NH * HD + NKV * HD].reshape(NKV, HD, DIM)
    wv_full = w_qkv[NH * HD + NKV * HD:].reshape(NKV, HD, DIM)

    in_maps = []
    for c in range(NCORES):
        wq_c = wq_full[c][:, perm, :].reshape(JQ, DIM)      # permuted q rows
        wk_c = wk_full[c][perm, :]                           # permuted k rows
        wv_c = wv_full[c]                                    # v unpermuted
        w_cat = np.concatenate([wq_c, wk_c, wv_c], axis=0)   # [768, DIM]
        # wo: [DIM, 512] -> [DIM, 4, 128] -> [128, DIM, 4] (1KB bf16 runs)
        wo_c = w_o[:, c * JQ:(c + 1) * JQ].reshape(DIM, HPG, 128)
        wo_c = np.ascontiguousarray(
            wo_c.transpose(2, 0, 1).astype(ml_dtypes.bfloat16))
        in_maps.append(dict(
            xt=np.ascontiguousarray(x.T),
            xpart=np.ascontiguousarray(x.T[c * JQ:(c + 1) * JQ]),
            wq=np.ascontiguousarray(w_cat.T),                # [DIM, 768]
            wo=wo_c,                                         # [128, DIM, 4]
            rms=np.ascontiguousarray(rms_w[c * JQ:(c + 1) * JQ]),
            cosf=cosf, sinf=sinf,
            maskt=maskb, ident=ident, onesc=onesc, onesr=onesr,
        ))
    return in_maps


def combine_outputs(results):
    """Sum per-core [DIM, T] f16 partials, dequant by c2, return [T, DIM]."""
    acc = np.zeros(results[0]["yt"].shape, dtype=np.float32)
    for r in results:
        acc += r["yt"].astype(np.float32)
    # c2o [NCH, 128(tl), KB(tb)] -> c2[t], t = c*CT + tb*128 + tl
    c2o = np.asarray(results[0]["c2o"], dtype=np.float32)
    c2 = c2o.transpose(0, 2, 1).reshape(-1)
    acc *= c2[None, :]
    return np.ascontiguousarray(acc.T)


def _install_axon_profile_shim():
    """Register antenv.axon_hooks NTFF hook missing from the agent image."""
    import types
    try:
        import antenv.axon_hooks  # noqa: F401
        return
    except ImportError:
        pass
    try:
        import antenv
        from trn_agent_boot.trn_boot import _ntff_profile_via_ctypes
    except ImportError:
        return
    so_path = "/opt/axon/libaxon_pjrt.so"
    import os
    if not os.path.exists(so_path):
        return
    mod = types.ModuleType("antenv.axon_hooks")
    _hook = {"fn": _ntff_profile_via_ctypes(so_path)}
    mod.set_axon_ntff_profile_hook = lambda fn: _hook.__setitem__("fn", fn)
    mod.get_axon_ntff_profile_hook = lambda: _hook["fn"]
    sys.modules["antenv.axon_hooks"] = mod
    antenv.axon_hooks = mod


_install_axon_profile_shim()


# ======================= public entry point =============================

_NC_CACHE = {}


def _get_nc(T):
    if T not in _NC_CACHE:
        _NC_CACHE[T] = build_kernel(T=T)
    return _NC_CACHE[T]


def kernel(x, w_qkv, w_o, rms_w, cache_k=None, cache_v=None, **_ignored):
    """Full-input entry: shards across 8 NeuronCores, returns [T, DIM] f32.

    cache_k/cache_v are accepted for signature compatibility; the module
    overwrites all T positions, so their (zero) contents are irrelevant.
    """
    from concourse.bass_utils import run_bass_kernel_spmd
    x = np.asarray(x, dtype=np.float32)
    w_qkv = np.asarray(w_qkv, dtype=np.float32)
    w_o = np.asarray(w_o, dtype=np.float32)
    rms_w = np.asarray(rms_w, dtype=np.float32)
    T = x.shape[0]
    nc = _get_nc(T)
    in_maps = make_inputs(x, w_qkv, w_o, rms_w, T=T)
    res = run_bass_kernel_spmd(nc, in_maps, core_ids=list(range(NCORES)))
    return combine_outputs(res.results)


def kernel_profiled(x, w_qkv, w_o, rms_w, cache_k=None, cache_v=None):
    """Like kernel() but with NTFF tracing; returns (y, exec_time_ns)."""
    from concourse.bass_utils import run_bass_kernel_spmd
    T = np.asarray(x).shape[0]
    nc = _get_nc(T)
    in_maps = make_inputs(np.asarray(x, np.float32), np.asarray(w_qkv, np.float32),
                          np.asarray(w_o, np.float32), np.asarray(rms_w, np.float32),
                          T=T)
    res = run_bass_kernel_spmd(nc, in_maps, core_ids=list(range(NCORES)),
                               trace=True)
    return combine_outputs(res.results), res.exec_time_ns


# revision 58
# speedup vs baseline: 1.0217x; 1.0217x over previous
#!/usr/bin/env python
"""Tensor-parallel fused attention kernel for Trainium2 (8 NeuronCores).

Sharding: one KV head (+ its 4 grouped Q heads) per core.
 - w_qkv column-parallel (each core computes its 768 qkv rows)
 - attention fully local per core (its heads)
 - RMSNorm/quant per-token stats via a tiny per-chunk AllGather whose
   consumer chain is deferred one chunk (engine queues are in-order, so
   this keeps the collective latency off the critical path)
 - w_o row-parallel; partial [DIM, T] outputs (f16) summed on host.

V2 layout rules (vs the V1 baseline):
 - NO DRAM round-trips for transposes/broadcasts: partition reductions go
   through PE transposes, row->all-partition broadcasts through K=1
   matmuls with a ones [1,128] stationary.
 - elementwise work split across DVE / Activation / Pool engines.
 - w_o is streamed once at the end; qz for all T resident as bf16
   integers (exact: |qz_int| <= 127), the per-token dequant scale c2 is
   factored out of the matmul and applied to the [DIM,T] output tiles.
 - y partials written as f16 (halves the output DMA).

Per-core layouts (everything transposed, d-major):
 - xT [DIM, T]; per-sub xq [128, 4, CT] f32r (transient)
 - q/k head-dim rows permuted (evens then odds) so RoPE is a half-swap
   done with partition-offset DVE ops (no DMA); v unpermuted.
 - K_sb [128, T] f32r, V_sb token-major [128, T/128, 128] f32r
 - z (attn out * rms_w) [128, 4, CT] f32 double-buffered
 - wo DRAM [128, DIM, 4] bf16 so each [128,128] stationary slice is
   read with 1KB runs.
"""
import sys
sys.path.insert(0, '/opt/trn_rl_repo')

import numpy as np
from contextlib import ExitStack

import concourse.bass as bass
import concourse.bacc as bacc_mod
import concourse.tile as tile
import concourse.mybir as mybir

F32 = mybir.dt.float32
F32R = mybir.dt.float32r
F16 = mybir.dt.float16
BF16 = mybir.dt.bfloat16
AF = mybir.ActivationFunctionType
OP = mybir.AluOpType
AX = mybir.AxisListType

DIM = 4096
NH = 32
NKV = 8
HPG = 4          # q heads per kv head (per core)
HD = 128
NCORES = 8
JQ = HPG * HD    # 512 local q rows
JL = JQ + 2 * HD # 768 local qkv rows
CT = 256         # tokens per chunk
KB = CT // 128   # key blocks (128 tokens) per chunk
NSUB = 8         # x sub-loads per chunk (4 d-chunks each)
DSUB = 4         # d-chunks per sub-load
THETA = 500000.0
EPS = 1e-5
SCALE = float(HD) ** -0.5
MAGIC = float(3 << 22)  # 12582912.0 = 1.5*2^23, ulp 1.0 range
DCH = DIM // 128  # 32 d-chunks


def build_kernel(T=2048, use_cc=True, debug=False):
    NCH = T // CT
    nc = bacc_mod.Bacc("TRN2", num_devices=NCORES)
    if debug:
        dbg_qk = nc.dram_tensor("dbg_qk", [JL, T], F32, kind="ExternalOutput")
        dbg_z = nc.dram_tensor("dbg_z", [JQ, T], F32, kind="ExternalOutput")

    # ---- I/O -------------------------------------------------------------
    xt_d = nc.dram_tensor("xt", [DIM, T], F32, kind="ExternalInput")
    xpart_d = nc.dram_tensor("xpart", [JQ, T], F32, kind="ExternalInput")
    wq_d = nc.dram_tensor("wq", [DIM, JL], BF16, kind="ExternalInput")
    wo_d = nc.dram_tensor("wo", [128, DIM, HPG], BF16, kind="ExternalInput")
    rms_d = nc.dram_tensor("rms", [JQ], F32, kind="ExternalInput")
    cosf_d = nc.dram_tensor("cosf", [128, T], F32, kind="ExternalInput")
    sinf_d = nc.dram_tensor("sinf", [128, T], F32, kind="ExternalInput")
    mask_d = nc.dram_tensor("maskt", [128, KB, CT], BF16, kind="ExternalInput")
    id_d = nc.dram_tensor("ident", [128, 128], F32R, kind="ExternalInput")
    onesc_d = nc.dram_tensor("onesc", [128, 1], F32R, kind="ExternalInput")
    onesr_d = nc.dram_tensor("onesr", [1, 128], F32R, kind="ExternalInput")
    yt_d = nc.dram_tensor("yt", [DIM, T], F16, kind="ExternalOutput")
    c2o_d = nc.dram_tensor("c2o", [T // CT, 128, KB], F32, kind="ExternalOutput")

    with ExitStack() as ctx:
        tc = ctx.enter_context(tile.TileContext(nc))
        persist = ctx.enter_context(tc.tile_pool(name="persist", bufs=1))
        work = ctx.enter_context(tc.tile_pool(name="work", bufs=2))
        dram = ctx.enter_context(tc.tile_pool(name="dram", bufs=1, space="DRAM"))
        # PSUM pools: 8 banks x 2KB/partition; slots are bank-granular, and a
        # bank supports only ONE OPEN accumulation group at a time (verified
        # on HW: interleaving two open groups corrupts the first), though
        # sequential groups + sub-view hazards are fine.
        # pq0-5: 6 qkv accumulators, one bank each. The same six families
        # are reused (by tag) in the attention phase -- sp rotates pq0/pq1
        # (kb parity), pv rotates pq2/pq3 (head parity), lp uses pq4,
        # ivl broadcasts pq5 -- and yp reuses them in the tail.
        # misc (transposes, small broadcasts), bufs=2             = 2 banks
        mm_ps = ctx.enter_context(tc.tile_pool(name="mm_ps", bufs=1, space="PSUM"))
        misc_ps = ctx.enter_context(tc.tile_pool(name="misc_ps", bufs=2, space="PSUM"))

        # ---- persistent tiles -------------------------------------------
        wq_sb = persist.tile([128, DCH, JL], BF16)
        for pc in range(NSUB):
            nc.sync.dma_start(
                wq_sb[:, pc * DSUB:(pc + 1) * DSUB, :],
                wq_d.ap()[pc * DSUB * 128:(pc + 1) * DSUB * 128, :]
                .rearrange("(dc p) j -> p dc j", p=128))
        K_sb = persist.tile([128, T], F32R)
        V_sb = persist.tile([128, T // 128, HD], F32R)
        qz_all = persist.tile([128, HPG, T], BF16)   # round(z*qf): ints <=127
        rms_sb = persist.tile([128, HPG], F32)
        nc.sync.dma_start(rms_sb[:], rms_d.ap().rearrange("(h p) -> p h", p=128))
        id_sb = persist.tile([128, 128], F32R)
        nc.sync.dma_start(id_sb[:], id_d.ap())
        onesc_sb = persist.tile([128, 1], F32R)
        nc.sync.dma_start(onesc_sb[:], onesc_d.ap())
        onesr_sb = persist.tile([1, 128], F32R)
        nc.sync.dma_start(onesr_sb[:], onesr_d.ap())
        mask_sb = persist.tile([128, KB, CT], BF16)
        nc.sync.dma_start(mask_sb[:], mask_d.ap())
        eps_sb = persist.tile([128, 1], F32)
        nc.vector.memset(eps_sb[:], EPS)

        def bcast_row(row_ap, name):
            """[1, 128] f32 row -> [128, 128] PSUM tile via K=1 matmul."""
            ps = misc_ps.tile([128, 128], F32, name=name, tag="misc")
            nc.tensor.matmul(ps[:], onesr_sb[:].bitcast(F32), row_ap,
                             start=True, stop=True)
            return ps

        def post_chain(c, z_c, statg):
            """Deferred consumer of chunk c's stats AllGather: global stats,
            quant coefficients, and z -> qz_all/c2_all. Emitted one chunk
            later so the in-order engine queues never stall on the CC."""
            t0 = c * CT
            tsl = slice(t0, t0 + CT)
            gst = work.tile([128, NCORES, KB, 2], F32, name="gst", bufs=2)
            nc.sync.dma_start(gst[:],
                              statg[:].rearrange("c p b s -> p c b s"))
            sst = work.tile([128, KB], F32, name="sst", bufs=2)
            nc.vector.tensor_reduce(
                sst[:], gst[:, :, :, 0].rearrange("p c b -> p b c"),
                axis=AX.X, op=OP.add)
            mxt = work.tile([128, KB], F32, name="mxt", bufs=2)
            nc.vector.tensor_reduce(
                mxt[:], gst[:, :, :, 1].rearrange("p c b -> p b c"),
                axis=AX.X, op=OP.max)
            # r = 1/sqrt(ssq/DIM + eps); rg = clip(r*gmax); qf = 127*r/rg
            rc = work.tile([128, KB], F32, name="rc", bufs=2)
            nc.scalar.activation(rc[:], sst[:], AF.Sqrt, scale=1.0 / DIM,
                                 bias=eps_sb[:])
            nc.vector.reciprocal(rc[:], rc[:])
            rg = work.tile([128, KB], F32, name="rg", bufs=2)
            nc.vector.tensor_tensor(rg[:], rc[:], mxt[:], OP.mult)
            nc.vector.tensor_scalar_max(rg[:], rg[:], 1e-5)
            qc4 = work.tile([128, 2 * KB], F32, name="qc4", bufs=2)
            rr = work.tile([128, KB], F32, name="rr", bufs=2)
            nc.vector.reciprocal(rr[:], rg[:])
            nc.vector.scalar_tensor_tensor(qc4[:, 0:KB], rr[:], 127.0,
                                           rc[:], OP.mult, OP.mult)
            nc.vector.tensor_scalar_mul(qc4[:, KB:], rg[:], 1.0 / 127.0)
            # c2 goes to the host (applied to the summed partials there)
            nc.sync.dma_start(c2o_d.ap()[c], qc4[:, KB:])
            qt_ps = misc_ps.tile([1, KB * 128], F32, name="qt_ps", tag="misc")
            for j in range(KB):
                nc.tensor.transpose(qt_ps[0:1, j * 128:(j + 1) * 128],
                                    qc4[:, j:j + 1], id_sb[:].bitcast(F32))
            qrow = work.tile([1, KB * 128], F32, name="qrow", bufs=1)
            nc.vector.tensor_copy(qrow[:], qt_ps[:])
            qf_bc = work.tile([128, HPG, CT], F32, name="qf_bc", bufs=1)
            for tb in range(KB):
                ps = bcast_row(qrow[0:1, tb * 128:(tb + 1) * 128], "qfb_ps")
                for hh in range(HPG):
                    nc.scalar.activation(qf_bc[:, hh, tb * 128:(tb + 1) * 128],
                                         ps[:], AF.Copy)
            # quantize: qz_int = round(z*qf) (exact integers in bf16);
            # z*qf written in place (last use of z_c)
            qzv = qz_all[:, :, tsl]
            nc.vector.tensor_tensor(z_c[:], z_c[:], qf_bc[:], OP.mult)
            nc.scalar.activation(z_c[:], z_c[:], AF.Copy, bias=MAGIC)
            nc.scalar.activation(qzv, z_c[:], AF.Copy, bias=-MAGIC)

        def pass1_pre(c):
            """Local abs-max partials for chunk c + AllGather issue.
            Emitted TWO chunks ahead of the heavy block, so the collective
            completes during the previous block. The first two chunks
            compute the full max locally instead (no collective rendezvous
            on the cold-start critical path)."""
            t0 = c * CT
            tsl = slice(t0, t0 + CT)
            # Each core reduces only ITS 512 dims of x (the same rows its
            # attention heads own); partial maxima are exchanged via a tiny
            # AllGather that hides under the previous chunk's attention.
            xs = work.tile([128, DSUB, CT], F32, name="xs", bufs=3)
            nc.sync.dma_start(
                xs[:], xpart_d.ap()[:, tsl].rearrange("(d p) t -> p d t", p=128))
            mpart = work.tile([128, CT], F32, name="mpart", bufs=1)
            nc.vector.tensor_reduce(
                mpart[:], xs[:].rearrange("p d t -> p t d"), axis=AX.X,
                op=OP.max, apply_absolute_value=True)
            mcolp = work.tile([128, KB], F32, name="mcolp", bufs=2)
            for tb in range(KB):
                mt = misc_ps.tile([128, 128], F32, name="mt", tag="misc")
                nc.tensor.transpose(
                    mt[:], mpart[:, tb * 128:(tb + 1) * 128],
                    id_sb[:].bitcast(F32))
                nc.vector.tensor_reduce(mcolp[:, tb:tb + 1], mt[:], axis=AX.X,
                                        op=OP.max)
            mpd = dram.tile([128, KB], F32, name=f"mpd{c}")
            nc.sync.dma_start(mpd[:], mcolp[:])
            mpg = dram.tile([NCORES, 128, KB], F32, name=f"mpg{c}")
            if use_cc:
                nc.gpsimd.collective_compute(
                    "AllGather", OP.bypass, replica_groups=[list(range(NCORES))],
                    ins=[mpd[:].opt()], outs=[mpg[:].opt()])
            else:
                for cc in range(NCORES):
                    nc.sync.dma_start(mpg[cc], mpd[:])
            return ("cc", mpg)

        def pass1_post(c, token):
            """Consume chunk c's abs-max (AllGather or local) -> s/sinv
            broadcasts. Emitted one chunk ahead of the heavy block."""
            kind, val = token
            if kind == "cc":
                mg = work.tile([128, NCORES, KB], F32, name="mg", bufs=2)
                nc.sync.dma_start(mg[:], val[:].rearrange("c p b -> p c b"))
                mcol = work.tile([128, KB], F32, name="mcol", bufs=2)
                nc.vector.tensor_reduce(
                    mcol[:], mg[:].rearrange("p c b -> p b c"), axis=AX.X,
                    op=OP.max)
            else:
                mcol = val
            nc.vector.tensor_scalar_max(mcol[:], mcol[:], 1e-5)
            scoef = work.tile([128, 2 * KB], F32, name="scoef", bufs=2)
            rec = work.tile([128, KB], F32, name="rec", bufs=1)
            nc.vector.reciprocal(rec[:], mcol[:])
            nc.vector.tensor_scalar_mul(scoef[:, 0:KB], rec[:], 127.0)
            nc.vector.tensor_scalar_mul(scoef[:, KB:], mcol[:], 1.0 / 127.0)
            st_ps = misc_ps.tile([1, 2 * KB * 128], F32, name="st_ps", tag="misc")
            for j in range(2 * KB):
                nc.tensor.transpose(st_ps[0:1, j * 128:(j + 1) * 128],
                                    scoef[:, j:j + 1], id_sb[:].bitcast(F32))
            srow = work.tile([1, 2 * KB * 128], F32, name="srow", bufs=2)
            nc.vector.tensor_copy(srow[:], st_ps[:])
            # broadcast rows across partitions (PSUM), then copy to SBUF
            s_bc = work.tile([128, DSUB, CT], F32, name="s_bc", bufs=2)
            sinv_bc = work.tile([128, CT], F32, name="sinv_bc", bufs=2)
            for tb in range(KB):
                ps = bcast_row(srow[0:1, tb * 128:(tb + 1) * 128], "sbc_ps")
                for d in range(DSUB):
                    nc.scalar.activation(s_bc[:, d, tb * 128:(tb + 1) * 128],
                                         ps[:], AF.Copy)
                ps2 = bcast_row(srow[0:1, (KB + tb) * 128:(KB + tb + 1) * 128],
                                "svbc_ps")
                nc.scalar.activation(sinv_bc[:, tb * 128:(tb + 1) * 128], ps2[:],
                                     AF.Copy)
            return s_bc, sinv_bc

        def qkv_part(c, s_bc, sinv_bc):
            t0 = c * CT
            tsl = slice(t0, t0 + CT)
            cos_ch = work.tile([128, CT], F32, name="cos_ch", bufs=2)
            nc.sync.dma_start(cos_ch[:], cosf_d.ap()[:, tsl])
            sin_ch = work.tile([128, CT], F32, name="sin_ch", bufs=2)
            nc.sync.dma_start(sin_ch[:], sinf_d.ap()[:, tsl])

            # ---- pass 2 over x: quantize + QKV projection ---------------
            pq = [mm_ps.tile([128, CT], F32, name=f"pq{jc}", tag=f"pq{jc}")
                  for jc in range(6)]
            for sub in range(NSUB):
                xs2 = work.tile([128, DSUB, CT], F32, name="xs", bufs=3)
                nc.sync.dma_start(
                    xs2[:], xt_d.ap()[sub * DSUB * 128:(sub + 1) * DSUB * 128, tsl]
                    .rearrange("(d p) t -> p d t", p=128))
                xq = work.tile([128, DSUB, CT], BF16, name="xq", bufs=2)
                nc.vector.tensor_tensor(xs2[:], xs2[:], s_bc[:], OP.mult)
                nc.scalar.activation(xs2[:], xs2[:], AF.Copy, bias=MAGIC)
                nc.scalar.activation(xq[:], xs2[:], AF.Copy, bias=-MAGIC)
                for jc in range(6):
                    for di in range(DSUB):
                        dc = sub * DSUB + di
                        nc.tensor.matmul(
                            pq[jc][:], wq_sb[:, dc, jc * 128:(jc + 1) * 128],
                            xq[:, di, :], start=(dc == 0), stop=(dc == DCH - 1))

            # ---- sinv scale + split into q / k / v ----------------------
            q4 = work.tile([128, HPG, CT], F32R, name="q4", bufs=1)
            k_tmp = work.tile([128, CT], F32, name="k_tmp", bufs=1)
            v_tmp = work.tile([128, CT], F32R, name="v_tmp", bufs=2)
            for jc in range(6):
                dst = (q4[:, jc, :] if jc < HPG
                       else (k_tmp[:] if jc == HPG else v_tmp[:]))
                nc.vector.tensor_tensor(dst, pq[jc][:], sinv_bc[:], OP.mult)

            # ---- RoPE (half-swap via small SBUF-SBUF DMA) ---------------
            def rope(dst, src):
                xsw = work.tile([128, CT], F32, name="xsw", bufs=2)
                nc.sync.dma_start(xsw[0:64, :], src[64:128, :])
                nc.sync.dma_start(xsw[64:128, :], src[0:64, :])
                nc.vector.tensor_tensor(xsw[:], xsw[:], sin_ch[:], OP.mult)
                a = work.tile([128, CT], F32, name="rcos", bufs=1)
                nc.vector.tensor_tensor(a[:], src, cos_ch[:], OP.mult)
                nc.vector.tensor_tensor(dst, a[:], xsw[:], OP.add)

            for h in range(HPG):
                rope(q4[:, h, :], q4[:, h, :].bitcast(F32))
            rope(K_sb[:, tsl], k_tmp[:])
            if debug:
                for h in range(HPG):
                    nc.sync.dma_start(dbg_qk.ap()[h * 128:(h + 1) * 128, tsl],
                                      q4[:, h, :].bitcast(F32))
                nc.sync.dma_start(dbg_qk.ap()[JQ:JQ + HD, tsl],
                                  K_sb[:, tsl].bitcast(F32))
                nc.sync.dma_start(dbg_qk.ap()[JQ + HD:, tsl],
                                  v_tmp[:].bitcast(F32))

            # ---- V transpose to token-major -----------------------------
            for tb in range(KB):
                vt = misc_ps.tile([128, 128], F32R, name="vt", tag="misc")
                nc.tensor.transpose(vt[:], v_tmp[:, tb * 128:(tb + 1) * 128],
                                    id_sb[:])
                nc.vector.tensor_copy(V_sb[:, c * KB + tb, :], vt[:])

            return q4

        def attn_part(c, q4):
            t0 = c * CT
            tsl = slice(t0, t0 + CT)
            # ---- attention (per head; normalization deferred one head) --
            nkb = KB * (c + 1)
            z_c = work.tile([128, HPG, CT], F32, name="z_c", bufs=2)
            ssqmx = work.tile([128, KB, 2], F32, name="ssqmx", bufs=2)

            def finish_head(h, pv, lrow):
                # PE-broadcast of 1/l, then z = pv/l, ssq stats, rms scale
                ivl_ps = mm_ps.tile([128, CT], F32, name="ivl_ps", tag="pq5")
                nc.tensor.matmul(ivl_ps[:], onesr_sb[:].bitcast(F32), lrow[:],
                                 start=True, stop=True)
                ivl_sb = work.tile([128, CT], F32, name="ivl_sb", bufs=1)
                nc.scalar.activation(ivl_sb[:], ivl_ps[:], AF.Copy)
                nc.vector.tensor_tensor(z_c[:, h, :], pv[:], ivl_sb[:], OP.mult)
                sq = work.tile([128, CT], F32R, name="sq", bufs=1)
                nc.scalar.activation(sq[:], z_c[:, h, :], AF.Square)
                for tb in range(KB):
                    sqt = misc_ps.tile([128, 128], F32R, name="sqt", tag="misc")
                    nc.tensor.transpose(sqt[:], sq[:, tb * 128:(tb + 1) * 128],
                                        id_sb[:])
                    if h == 0:
                        nc.vector.tensor_reduce(ssqmx[:, tb, 0:1], sqt[:],
                                                axis=AX.X, op=OP.add)
                    else:
                        hcol = work.tile([128, 1], F32, name="hcol", bufs=2)
                        nc.vector.tensor_reduce(hcol[:], sqt[:], axis=AX.X,
                                                op=OP.add)
                        nc.vector.tensor_tensor(ssqmx[:, tb, 0:1],
                                                ssqmx[:, tb, 0:1], hcol[:],
                                                OP.add)
                # z = out * rms_w (per-partition scalar); in-place after Square
                nc.vector.tensor_scalar(z_c[:, h, :], z_c[:, h, :],
                                        rms_sb[:, h:h + 1], None, OP.mult)
                if debug:
                    nc.sync.dma_start(dbg_z.ap()[h * 128:(h + 1) * 128, tsl],
                                      z_c[:, h, :])

            prev_head = None
            for h in range(HPG):
                pv = mm_ps.tile([128, CT], F32, name="pv", tag=f"pq{2 + h % 2}")
                lp = mm_ps.tile([1, CT], F32, name="lp", tag="pq4")
                for kp in range(nkb // 2):
                    # two key blocks share one score bank and a single exp
                    sp = mm_ps.tile([128, 2, CT], F32, name="sp",
                                    tag=f"pq{kp % 2}")
                    for i in range(2):
                        kb = 2 * kp + i
                        nc.tensor.matmul(sp[:, i, :],
                                         K_sb[:, kb * 128:(kb + 1) * 128],
                                         q4[:, h, :], start=True, stop=True,
                                         skip_group_check=True)
                    P = work.tile([128, 2, CT], F32R, name="P", bufs=2)
                    nc.scalar.activation(P[:], sp[:], AF.Exp, scale=SCALE)
                    if kp == nkb // 2 - 1:
                        nc.gpsimd.tensor_tensor(P[:], P[:].bitcast(F32),
                                                mask_sb[:], OP.mult)
                    for i in range(2):
                        kb = 2 * kp + i
                        nc.tensor.matmul(pv[:], V_sb[:, kb, :], P[:, i, :],
                                         start=(kb == 0), stop=(kb == nkb - 1))
                        nc.tensor.matmul(lp[:], onesc_sb[:], P[:, i, :],
                                         start=(kb == 0), stop=(kb == nkb - 1))
                lrow = work.tile([1, CT], F32, name="lrow", bufs=2)
                nc.vector.reciprocal(lrow[:], lp[:])
                if prev_head is not None:
                    finish_head(*prev_head)
                prev_head = (h, pv, lrow)
            finish_head(*prev_head)

            # ---- per-token max|z| over local dims -----------------------
            mz = work.tile([128, CT], F32, name="mz", bufs=2)
            nc.vector.tensor_reduce(
                mz[:], z_c[:].rearrange("p h t -> p t h"),
                axis=AX.X, op=OP.max, apply_absolute_value=True)
            for tb in range(KB):
                mzt = misc_ps.tile([128, 128], F32, name="mzt", tag="misc")
                nc.tensor.transpose(mzt[:], mz[:, tb * 128:(tb + 1) * 128],
                                    id_sb[:].bitcast(F32))
                nc.vector.tensor_reduce(ssqmx[:, tb, 1:2], mzt[:], axis=AX.X,
                                        op=OP.max)

            # ---- stats collective (consumed one chunk later) ------------
            statd = dram.tile([128, KB, 2], F32, name=f"statd{c}")
            nc.sync.dma_start(statd[:], ssqmx[:])
            statg = dram.tile([NCORES, 128, KB, 2], F32, name=f"statg{c}")
            if use_cc:
                nc.gpsimd.collective_compute(
                    "AllGather", OP.bypass, replica_groups=[list(range(NCORES))],
                    ins=[statd[:].opt()], outs=[statg[:].opt()])
            else:
                for cc in range(NCORES):
                    nc.sync.dma_start(statg[cc], statd[:])
            return z_c, statg

        # ---- software-pipelined main loop ------------------------------
        # iteration i emits, in order: QKV(i-2) (so xq production leads the
        # PE), max-collective issue(i), collective consume(i-1) -> s/sinv,
        # attention(i-2), stats chain(i-3).
        pending = None
        mpg_carry = None
        sc_carry = None
        for i in range(NCH + 2):
            mpg_nxt = pass1_pre(i) if i < NCH else None
            sc_nxt = pass1_post(i - 1, mpg_carry) if 1 <= i <= NCH else None
            if i >= 2:
                q4h = qkv_part(i - 2, *sc_carry)
                z_c, statg = attn_part(i - 2, q4h)
                if pending is not None:
                    post_chain(*pending)
                pending = (i - 2, z_c, statg)
            mpg_carry = mpg_nxt
            sc_carry = sc_nxt

        post_chain(*pending)

        # ---- tail: single streamed w_o pass ------------------------------
        NTS = T // CT
        for ic in range(DCH):
            wo_t = work.tile([128, 128, HPG], BF16, name="wo_t", bufs=2)
            nc.sync.dma_start(wo_t[:], wo_d.ap()[:, ic * 128:(ic + 1) * 128, :])
            for half in range(4):
                yrow = work.tile([128, T // 4], F16, name="yrow", bufs=2)
                nts_h = NTS // 4
                yps = [mm_ps.tile([128, CT], F32, name=f"yp{tsi}",
                                  tag=f"pq{2 * (half % 2) + tsi}")
                       for tsi in range(nts_h)]
                for jc in range(HPG):
                    for tsi in range(nts_h):
                        ts = half * nts_h + tsi
                        nc.tensor.matmul(
                            yps[tsi][:], wo_t[:, :, jc],
                            qz_all[:, jc, ts * CT:(ts + 1) * CT],
                            start=(jc == 0), stop=(jc == HPG - 1))
                # y = wo @ qz_int (the c2 dequant is applied on the host)
                for tsi in range(nts_h):
                    dst = yrow[:, tsi * CT:(tsi + 1) * CT]
                    if tsi % 2 == 0:
                        nc.vector.tensor_copy(dst, yps[tsi][:])
                    else:
                        nc.scalar.activation(dst, yps[tsi][:], AF.Copy)
                nc.sync.dma_start(
                    yt_d.ap()[ic * 128:(ic + 1) * 128,
                              half * (T // 4):(half + 1) * (T // 4)],
                    yrow[:])
    nc.compile()
    return nc


# ======================= host-side preparation ==========================

def _rope_tables(T):
    import jax
    import jax.numpy as jnp
    cpu = jax.devices("cpu")[0]
    with jax.default_device(cpu):
        inv = THETA ** (-jnp.arange(0, HD, 2, dtype=jnp.float32) / HD)
        pos = jnp.arange(T, dtype=jnp.float32)
        ang = pos[None, :] * inv[:, None]          # [64, T]
        cos = np.asarray(jnp.cos(ang), dtype=np.float32)
        sin = np.asarray(jnp.sin(ang), dtype=np.float32)
    cosf = np.concatenate([cos, cos], axis=0)       # [128, T]
    sinf = np.concatenate([-sin, sin], axis=0)
    return np.ascontiguousarray(cosf), np.ascontiguousarray(sinf)


def _perm_rope():
    """head-dim permutation: evens then odds."""
    return np.concatenate([np.arange(0, HD, 2), np.arange(1, HD, 2)])


def make_inputs(x, w_qkv, w_o, rms_w, T=2048):
    """Build the 8 per-core input dicts from full inputs."""
    import ml_dtypes
    perm = _perm_rope()
    cosf, sinf = _rope_tables(T)
    mask = np.zeros((128, KB, CT), dtype=np.float32)
    kt = np.arange(128)[:, None]
    qt = np.arange(CT)[None, :]
    for d in range(KB):
        mask[:, d, :] = (kt + 128 * d <= qt)
    maskb = mask.astype(ml_dtypes.bfloat16)
    ident = np.eye(128, dtype=np.float32)
    onesc = np.ones((128, 1), dtype=np.float32)
    onesr = np.ones((1, 128), dtype=np.float32)

    wq_full = w_qkv[:NH * HD].reshape(NKV, HPG, HD, DIM)
    wk_full = w_qkv[NH * HD:NH * HD + NKV * HD].reshape(NKV, HD, DIM)
    wv_full = w_qkv[NH * HD + NKV * HD:].reshape(NKV, HD, DIM)

    in_maps = []
    for c in range(NCORES):
        wq_c = wq_full[c][:, perm, :].reshape(JQ, DIM)      # permuted q rows
        wk_c = wk_full[c][perm, :]                           # permuted k rows
        wv_c = wv_full[c]                                    # v unpermuted
        w_cat = np.concatenate([wq_c, wk_c, wv_c], axis=0)   # [768, DIM]
        # wo: [DIM, 512] -> [DIM, 4, 128] -> [128, DIM, 4] (1KB bf16 runs)
        wo_c = w_o[:, c * JQ:(c + 1) * JQ].reshape(DIM, HPG, 128)
        wo_c = np.ascontiguousarray(
            wo_c.transpose(2, 0, 1).astype(ml_dtypes.bfloat16))
        in_maps.append(dict(
            xt=np.ascontiguousarray(x.T),
            xpart=np.ascontiguousarray(x.T[c * JQ:(c + 1) * JQ]),
            wq=np.ascontiguousarray(w_cat.T.astype(ml_dtypes.bfloat16)),
            wo=wo_c,                                         # [128, DIM, 4]
            rms=np.ascontiguousarray(rms_w[c * JQ:(c + 1) * JQ]),
            cosf=cosf, sinf=sinf,
            maskt=maskb, ident=ident, onesc=onesc, onesr=onesr,
        ))
    return in_maps


def combine_outputs(results):
    """Sum per-core [DIM, T] f16 partials, dequant by c2, return [T, DIM]."""
    acc = np.zeros(results[0]["yt"].shape, dtype=np.float32)
    for r in results:
        acc += r["yt"].astype(np.float32)
    # c2o [NCH, 128(tl), KB(tb)] -> c2[t], t = c*CT + tb*128 + tl
    c2o = np.asarray(results[0]["c2o"], dtype=np.float32)
    c2 = c2o.transpose(0, 2, 1).reshape(-1)
    acc *= c2[None, :]
    return np.ascontiguousarray(acc.T)


def _install_axon_profile_shim():
    """Register antenv.axon_hooks NTFF hook missing from the agent image."""
    import types
    try:
        import antenv.axon_hooks  # noqa: F401
        return
    except ImportError:
        pass
    try:
        import antenv
        from trn_agent_boot.trn_boot import _ntff_profile_via_ctypes
    except ImportError:
        return
    so_path = "/opt/axon/libaxon_pjrt.so"
    import os
    if not os.path.exists(so_path):
        return
    mod = types.ModuleType("antenv.axon_hooks")
    _hook = {"fn": _ntff_profile_via_ctypes(so_path)}
    mod.set_axon_ntff_profile_hook = lambda fn: _hook.__setitem__("fn", fn)
    mod.get_axon_ntff_profile_hook = lambda: _hook["fn"]
    sys.modules["antenv.axon_hooks"] = mod
    antenv.axon_hooks = mod


_install_axon_profile_shim()


# ======================= public entry point =============================

_NC_CACHE = {}


def _get_nc(T):
    if T not in _NC_CACHE:
        _NC_CACHE[T] = build_kernel(T=T)
    return _NC_CACHE[T]


def kernel(x, w_qkv, w_o, rms_w, cache_k=None, cache_v=None, **_ignored):
    """Full-input entry: shards across 8 NeuronCores, returns [T, DIM] f32.

    cache_k/cache_v are accepted for signature compatibility; the module
    overwrites all T positions, so their (zero) contents are irrelevant.
    """
    from concourse.bass_utils import run_bass_kernel_spmd
    x = np.asarray(x, dtype=np.float32)
    w_qkv = np.asarray(w_qkv, dtype=np.float32)
    w_o = np.asarray(w_o, dtype=np.float32)
    rms_w = np.asarray(rms_w, dtype=np.float32)
    T = x.shape[0]
    nc = _get_nc(T)
    in_maps = make_inputs(x, w_qkv, w_o, rms_w, T=T)
    res = run_bass_kernel_spmd(nc, in_maps, core_ids=list(range(NCORES)))
    return combine_outputs(res.results)


def kernel_profiled(x, w_qkv, w_o, rms_w, cache_k=None, cache_v=None):
    """Like kernel() but with NTFF tracing; returns (y, exec_time_ns)."""
    from concourse.bass_utils import run_bass_kernel_spmd
    T = np.asarray(x).shape[0]
    nc = _get_nc(T)
    in_maps = make_inputs(np.asarray(x, np.float32), np.asarray(w_qkv, np.float32),
                          np.asarray(w_o, np.float32), np.asarray(rms_w, np.float32),
                          T=T)
    res = run_bass_kernel_spmd(nc, in_maps, core_ids=list(range(NCORES)),
                               trace=True)
    return combine_outputs(res.results), res.exec_time_ns
